# revision 30
# baseline (speedup 1.0000x reference)
"""HTSubTree forward as a distributed Bass kernel on 8 TRN2 NeuronCores.

out[b,u,v,r] = sum_{i,j,p} x[b,(i,j)] * WL[i,u,p] * WR2[j,v,p,r]
  where WL = f0*f1*c_left and WR2 = f2*f3*c_right*c_root, both
  precontracted on host (tiny). Pure batch data-parallelism: 64 of 512
  batch elements per core.

v2 design (all matmuls bf16, fp32 PSUM accumulation):
  x2 [128, 2048]: partitions (pp,i), free (g, b2, j) — two pairs per
    group g stacked on the partition axis so stage1 runs as TWO
    concurrent K=64 row-tiled matmuls (tile_position (0,0)/(64,0)):
      pyX[(b2,j), (par,c,u)] = x_pair[i,(b2,j)].T @ wlf[i,(par,c,u)]
  evac: PSUM->SBUF cast copies to yS bf16 (DVE/ACT balanced)
  relayout (DVE bf16 SBUF->SBUF, 2x/4x perf mode):
      y2[(par,j), (pp,c,b2,u)] <- yS[(b2,j), (pp,par,c,u)]
  stage2 (x4 accum, K=128): po[(b2,u),(v,r)] += y2[:,pp,c].T @ wr2[c]
  out: PSUM->SBUF bf16 cast copy, 128KB DMA per pair; host upcasts.
"""

import sys

sys.path.insert(0, "/opt/trn_rl_repo")

import numpy as np
import ml_dtypes

import concourse.bass as bass
import concourse.tile as tile
from concourse import bacc, mybir
from concourse.bass_utils import run_bass_kernel_spmd

NCORES = 8
B = 512
BLOC = B // NCORES  # 64 batch elements per core
F32 = mybir.dt.float32
BF16 = mybir.dt.bfloat16
NPBF16 = ml_dtypes.bfloat16

_COMPILED = None


def _build():
    nc = bacc.Bacc("TRN2", target_bir_lowering=False, debug=False)
    x_ap = nc.dram_tensor("x2", [128, 2048], BF16, kind="ExternalInput").ap()
    wlf_ap = nc.dram_tensor("wlf2", [128, 512], BF16, kind="ExternalInput").ap()
    wr2_ap = nc.dram_tensor("wr2b", [4, 128, 512], BF16, kind="ExternalInput").ap()
    out_ap = nc.dram_tensor("out", [BLOC * 64, 512], BF16, kind="ExternalOutput").ap()

    # running engine-load estimates (ns) for DVE ("v") / ACT ("s") copy split
    load = {"v": 0.0, "s": 0.0}

    with tile.TileContext(nc) as tc:
        with (
            tc.tile_pool(name="weights", bufs=1) as wpool,
            tc.tile_pool(name="xin", bufs=1) as xpool,
            tc.tile_pool(name="ystage", bufs=4) as spool,
            tc.tile_pool(name="y2p", bufs=4) as y2pool,
            tc.tile_pool(name="ostage", bufs=3) as opool,
            tc.tile_pool(name="py", bufs=2, space="PSUM") as pypool,
            tc.tile_pool(name="po", bufs=4, space="PSUM") as popool,
        ):
            def psum_copy(dst, src):
                # [128,512] f32 PSUM -> bf16 SBUF: ~687ns on either engine
                if load["v"] + 687 <= load["s"] + 687:
                    load["v"] += 687
                    nc.vector.tensor_copy(dst, src)
                else:
                    load["s"] += 687
                    nc.scalar.copy(dst, src)

            # input DMAs, in gating order: tiny first-group x slice so
            # stage1 can start ASAP, then wlf, first stage2 weight chunk,
            # the bulk of x, then the remaining stage2 weights
            # first x slice covers groups 0-3 at 1KB/partition descriptors
            # (256B descriptors of a smaller slice fall off DMA line-rate)
            x0 = xpool.tile([128, 512], BF16, tag="xg0")
            nc.sync.dma_start(x0[:], x_ap[:, 0:512])
            wlf = wpool.tile([128, 512], BF16, tag="wlf")
            nc.sync.dma_start(wlf[:], wlf_ap[:])
            wr2 = []
            for c in range(4):
                wr2.append(wpool.tile([128, 512], BF16, tag=f"wr2c{c}",
                                      name=f"wr2c{c}"))
            xt = xpool.tile([128, 1536], BF16, tag="xrest")
            nc.sync.dma_start(xt[:], x_ap[:, 512:2048])
            for c in range(4):
                nc.sync.dma_start(wr2[c][:], wr2_ap[c])

            # HAM warmup: dummy matmuls on memset scratch keep the PE busy
            # while the x0/wlf DMAs land, then junk pads (below) carry the
            # busy-window through the pipeline-fill gaps so the clock-gate
            # reaches 8/8 early; results go to a junk PSUM tile, never read
            scratch = wpool.tile([64, 512], BF16, tag="warm_src")
            nc.gpsimd.memset(scratch[:], 0.0)
            warm_po = popool.tile([128, 512], F32, tag="po", space="PSUM",
                                  name="warm_po")

            def junk_mm(n):
                for _ in range(n):
                    nc.tensor.matmul(warm_po[:], scratch[:, 0:128],
                                     scratch[:], start=True, stop=True,
                                     tile_position=(0, 0))

            junk_mm(9)

            def stage1(g):
                # two concurrent row-tiled K=64 matmuls into one 2-bank
                # PSUM tile (pair pp=0 -> bank 0, pp=1 -> bank 1)
                if g < 4:
                    base = g * 128
                    xA, xB = x0[0:64, base:base + 128], x0[64:128, base:base + 128]
                else:
                    base = g * 128 - 512
                    xA, xB = xt[0:64, base:base + 128], xt[64:128, base:base + 128]
                pyA = pypool.tile([128, 512], F32, tag="pyA", space="PSUM")
                pyB = pypool.tile([128, 512], F32, tag="pyB", space="PSUM")
                nc.tensor.matmul(pyA[:], xA, wlf[0:64, :], start=True,
                                 stop=True, tile_position=(0, 0))
                nc.tensor.matmul(pyB[:], xB, wlf[64:128, :], start=True,
                                 stop=True, tile_position=(64, 0))
                # evac to bf16: yS free = pp*512 + par*256 + c*64 + u
                yS = spool.tile([128, 1024], BF16, tag="yS")
                psum_copy(yS[:, 0:512], pyA[:])
                psum_copy(yS[:, 512:1024], pyB[:])
                # relayout: y2 free = pp*512 + c*128 + b2*64 + u,
                # partitions (par,j); DVE bf16 fast path (ACT helps on the
                # first groups to shorten the pipeline-fill)
                y2 = y2pool.tile([128, 1024], BF16, tag="y2")
                srcv = yS.rearrange("q (pp par c u) -> par q pp c u",
                                    pp=2, par=2, c=4, u=64)
                dstv = y2.rearrange("q (pp c b2 u) -> b2 q pp c u",
                                    pp=2, c=4, b2=2, u=64)
                for b2 in range(2):
                    for par in range(2):
                        src = srcv[par][b2 * 64:(b2 + 1) * 64]
                        dst = dstv[b2][par * 64:(par + 1) * 64]
                        load["v"] += 288
                        nc.vector.tensor_copy(dst, src)
                return y2

            # software-pipelined: stage1/evac/relayout of g+1 are emitted
            # before stage2 of g, so the copies overlap stage2 streaming
            # and never block behind stage2-dependent out-copies in the
            # engine FIFOs
            y2s = {0: stage1(0), 1: stage1(1)}
            for g in range(16):
                if g + 2 < 16:
                    y2s[g + 2] = stage1(g + 2)
                y2 = y2s.pop(g)
                ot = opool.tile([128, 1024], BF16, tag="ot")
                for pp in range(2):
                    po = popool.tile([128, 512], F32, tag="po", space="PSUM")
                    for c in range(4):
                        nc.tensor.matmul(
                            po[:],
                            y2[:, pp * 512 + c * 128: pp * 512 + (c + 1) * 128],
                            wr2[c][:], start=(c == 0), stop=(c == 3))
                    psum_copy(ot[:, pp * 512:(pp + 1) * 512], po[:])
                    if g == 15:
                        # split the final group's DMA per pair so the pp0
                        # half drains during pp1's stage2 (shorter tail)
                        row = 256 * g + 128 * pp
                        nc.sync.dma_start(out_ap[row:row + 128, :],
                                          ot[:, pp * 512:(pp + 1) * 512])
                if g < 15:
                    dst = out_ap[256 * g: 256 * (g + 1), :].rearrange(
                        "(pp q) vr -> q pp vr", pp=2, q=128)
                    src = ot.rearrange("q (pp vr) -> q pp vr", pp=2, vr=512)
                    nc.sync.dma_start(dst, src)

    nc.compile()
    return nc


def _host_prep(x, factors, cores):
    """Pre-contract the tiny parameters and lay out per-core shards."""
    f0, f1, f2, f3 = factors[0], factors[1], factors[2], factors[3]
    c_root, c_left, c_right = cores[0], cores[1], cores[2]
    # WL[(i0,i1),(o0,o1),p=r02]
    wl = np.einsum("ioa,jpb,abr->ijopr", f0, f1, c_left, optimize=True)
    wl = wl.reshape(64, 64, 8)  # [i, u, p]
    # WRq[(i2,i3),(o2,o3),q=r24];  WR2[j,v,p,r] = sum_q WRq * c_root[p,q,r]
    wrq = np.einsum("ioc,jpd,cdq->ijopq", f2, f3, c_right, optimize=True).reshape(64, 64, 8)
    wr2 = np.einsum("jvq,pqr->jvpr", wrq, c_root, optimize=True)  # [j, v, p, r]

    # wlf [64, 512]: free = par*256 + c*64 + u  with  p = 2c + par; rows doubled
    wlf = np.ascontiguousarray(
        wl.reshape(64, 64, 4, 2).transpose(0, 3, 2, 1).reshape(64, 512))
    wlf2 = np.concatenate([wlf, wlf], axis=0).astype(NPBF16)
    # wr2c [4, 128, 512]: [c][par*64+j][v*8+r] = wr2[j, v, 2c+par, r]
    wr2c = np.ascontiguousarray(
        wr2.transpose(2, 0, 1, 3).reshape(4, 2, 64, 64, 8).reshape(4, 128, 512)
    ).astype(NPBF16)

    xf = x.reshape(B, 64, 64)
    xs = []
    for core in range(NCORES):
        xl = xf[core * BLOC:(core + 1) * BLOC]  # [64(b), 64(i), 64(j)]
        # x2[pp*64+i, g*128 + b2*64 + j] = xl[4g+2pp+b2, i, j]
        x2 = xl.reshape(16, 2, 2, 64, 64).transpose(1, 3, 0, 2, 4).reshape(128, 2048)
        xs.append(np.ascontiguousarray(x2).astype(NPBF16))
    return xs, wlf2, wr2c


def kernel(x, factors, cores, _want_profile=False):
    global _COMPILED
    x = np.asarray(x, dtype=np.float32)
    factors = np.asarray(factors, dtype=np.float32)
    cores = np.asarray(cores, dtype=np.float32)
    if _COMPILED is None:
        _COMPILED = _build()
    nc = _COMPILED
    xs, wlf2, wr2c = _host_prep(x, factors, cores)
    in_maps = [{"x2": xs[c], "wlf2": wlf2, "wr2b": wr2c} for c in range(NCORES)]
    res = run_bass_kernel_spmd(nc, in_maps, list(range(NCORES)), trace=_want_profile)
    out = np.concatenate(
        [np.asarray(res.results[c]["out"], dtype=np.float32).reshape(
            BLOC, 8, 8, 8, 8, 8) for c in range(NCORES)]
    )
    if _want_profile:
        return out, res
    return out
